# revision 2
# baseline (speedup 1.0000x reference)
"""Bahdanau-style cosine attention kernel for Trainium2 (8 NeuronCores).

reference math (fp32):
    q = squeeze(query)              # [H]
    dots = keys @ q                 # [S]
    cos = dots / (|q| * |keys_i|)   # [S]
    context = sum_i cos_i * keys_i  # [H]

Strategy (v2, bf16):
  - keys are cast to bf16 on the host (tolerance is 2e-2; bf16 input
    precision contributes ~5e-3) and sharded along S across 8 cores
    (4096 rows each); q is normalized by |q| on the host, cast to bf16,
    broadcast to 128 partitions. Host sums the 8 partial contexts.
  - HBM traffic per core: 8.4 MiB (vs 16.8 for fp32) -> ~22 us stream.
  - SBUF layout "(p t) c": partition p holds rows p*T..p*T+31 so each
    DMA chunk is one contiguous run per partition (row permutation is
    irrelevant: all reductions are permutation-invariant).
  - Per row-tile [128, 1024]: dots = DVE/GPS scalar_tensor_tensor accum;
    nrm2 = ACT Square accum (PSUM scratch) or GPS STT; these are 1x-rate
    ops so the 64 reduce passes are split across DVE/ACT/GPS to balance.
  - cos = dots * 1/sqrt(nrm2): ACT Sqrt + DVE reciprocal + DVE mul (bf16
    out), per group of tiles, high priority (gates the PE).
  - context += cos_t^T @ K_t on PE (bf16, 1 cyc/row, PSUM accumulate),
    folded out at the end via ACT copy + DMA.
"""

import os
import sys

import numpy as np

for _p in ("/opt/trn_rl_repo",):
    if os.path.isdir(_p) and _p not in sys.path:
        sys.path.append(_p)

P = 128          # SBUF partitions
H = 1024         # feature dim
S_FULL = 32768   # full sequence
N_CORES = 8
S = S_FULL // N_CORES   # rows per core = 4096
T = S // P              # row-tiles per core = 32

# DMA chunk sizes in tiles; front-loaded small so compute starts early,
# small tail so the last tiles land as soon as possible.
CHUNKS = [1, 1, 2, 4, 4, 4, 4, 4, 4, 2, 1, 1]
assert sum(CHUNKS) == T

# Engine placement per tile: dots engine and squares engine.
# 'V' = DVE (scalar_tensor_tensor), 'A' = ACT (Square w/ accum).
# GpSimd (Pool) cannot run TensorScalarPtr (walrus engine check), so the
# 64 reduce passes are split across DVE and ACT only: DVE takes all 32
# dots; ACT takes 31 squares (measured ACT ~1.33us/sq vs DVE ~1.31 --
# one square moves to DVE to even the finish).
DOTS_ENG = {_t: "V" for _t in range(T)}
SQ_ENG = {_t: "A" for _t in range(T)}
SQ_ENG[16] = "V"

# cos-group boundaries (tiles whose cos chain runs together)
GROUPS = [(0, 1), (1, 2), (2, 4), (4, 8), (8, 12),
          (12, 16), (16, 20), (20, 24), (24, 28), (28, 30), (30, 31),
          (31, 32)]

PE_WARMUP_MMS = 5

_NC_CACHE = {}


def _build_nc():
    import concourse.bacc as bacc
    import concourse.tile as tile
    from concourse import mybir

    f32 = mybir.dt.float32
    bf16 = mybir.dt.bfloat16
    AF = mybir.ActivationFunctionType
    OP = mybir.AluOpType
    nc = bacc.Bacc("TRN2", target_bir_lowering=False, debug=False)

    keys_d = nc.dram_tensor("keys", [S, H], bf16, kind="ExternalInput").ap()
    qb_d = nc.dram_tensor("qb", [P, H], bf16, kind="ExternalInput").ap()
    ctx_d = nc.dram_tensor("ctx", [1, H], f32, kind="ExternalOutput").ap()

    with tile.TileContext(nc) as tc:
        with (
            tc.tile_pool(name="main", bufs=1) as pool,
            tc.tile_pool(name="psum", bufs=1, space="PSUM") as pp,
        ):
            # both ACT table-set loads (Square's set, then the Sqrt set)
            # fire during the DMA prologue instead of stalling the dense
            # phase: dummy [1,1] activations with no data dependencies
            sqwarm = pool.tile([1, 2], f32, name="sqwarm")
            nc.vector.memset(sqwarm[:], 1.0)
            nc.scalar.activation(sqwarm[:, 0:1], sqwarm[:, 0:1], AF.Square)
            nc.scalar.activation(sqwarm[:, 1:2], sqwarm[:, 1:2], AF.Sqrt)

            qb = pool.tile([P, H], bf16, name="qb_sb")
            nc.sync.dma_start(qb[:], qb_d[:])

            # keys[p*T + t, c] -> sbuf[p, t, c]; contiguous per partition
            keys_r = keys_d.rearrange("(p t) c -> p t c", p=P)
            kcs = []
            t0 = 0
            for j, ct in enumerate(CHUNKS):
                kc = pool.tile([P, ct * H], bf16, name=f"kc{j}", tag=f"kc{j}")
                nc.sync.dma_start(kc[:], keys_r[:, t0:t0 + ct, :])
                kcs.append((kc, t0, ct))
                t0 += ct

            tile_of = {}
            for kc, t0, ct in kcs:
                for i in range(ct):
                    tile_of[t0 + i] = (kc, i)

            def ktile(t):
                kc, i = tile_of[t]
                return kc[:, i * H:(i + 1) * H]

            # PE warmup during DMA prologue (HAM clock ramp)
            ps_w = pp.tile([1, 512], f32, name="ps_w")
            for _ in range(PE_WARMUP_MMS):
                nc.tensor.matmul(ps_w[:], qb[:, 0:1], qb[:, 0:512],
                                 start=True, stop=True)

            dots = pool.tile([P, T], f32, name="dots")
            nrm2 = pp.tile([P, T], f32, name="nrm2")
            skewacc = pp.tile([P, 1], f32, name="skewacc")
            knrm = pool.tile([P, T], f32, name="knrm")
            rkn = pool.tile([P, T], f32, name="rkn")
            cosv = pool.tile([P, T], bf16, name="cosv")
            dvescr = pool.tile([P, H], bf16, name="dvescr")
            gpsscr = pool.tile([P, H], bf16, name="gpsscr")
            actscr = pp.tile([P, H], f32, name="actscr")
            ps0 = pp.tile([1, 512], f32, name="ps0")
            ps1 = pp.tile([1, 512], f32, name="ps1")

            # skew ACT ~1 tile behind DVE: when both engines stream the
            # same kc tile concurrently, SBUF read contention slows every
            # pass ~15-20% (v2 ran formula-exact only because its lazy
            # mid-stream table load created this skew by accident)
            nc.scalar.activation(actscr[:], qb[:], AF.Square,
                                 accum_out=skewacc[:])

            for gi, (g0, g1) in enumerate(GROUPS):
                for t in range(g0, g1):
                    kt = ktile(t)
                    if DOTS_ENG[t] == "V":
                        nc.vector.scalar_tensor_tensor(
                            out=dvescr[:], in0=kt, scalar=1.0, in1=qb[:],
                            op0=OP.mult, op1=OP.mult,
                            accum_out=dots[:, t:t + 1])
                    else:
                        nc.gpsimd.scalar_tensor_tensor(
                            out=gpsscr[:], in0=kt, scalar=1.0, in1=qb[:],
                            op0=OP.mult, op1=OP.mult,
                            accum_out=dots[:, t:t + 1])
                    if SQ_ENG[t] == "A":
                        nc.scalar.activation(
                            actscr[:], kt, AF.Square,
                            accum_out=nrm2[:, t:t + 1])
                    elif SQ_ENG[t] == "G":
                        nc.gpsimd.scalar_tensor_tensor(
                            out=gpsscr[:], in0=kt, scalar=1.0, in1=kt,
                            op0=OP.mult, op1=OP.mult,
                            accum_out=nrm2[:, t:t + 1])
                    else:
                        nc.vector.scalar_tensor_tensor(
                            out=dvescr[:], in0=kt, scalar=1.0, in1=kt,
                            op0=OP.mult, op1=OP.mult,
                            accum_out=nrm2[:, t:t + 1])
                cols = slice(g0, g1)
                with tc.high_priority(offset=40):
                    nc.scalar.activation(knrm[:, cols], nrm2[:, cols],
                                         AF.Sqrt)
                    nc.vector.reciprocal(rkn[:, cols], knrm[:, cols])
                    nc.vector.tensor_mul(cosv[:, cols], dots[:, cols],
                                         rkn[:, cols])
                for t in range(g0, g1):
                    kt = ktile(t)
                    nc.tensor.matmul(ps0[:], cosv[:, t:t + 1], kt[:, 0:512],
                                     start=(t == 0), stop=(t == T - 1))
                    nc.tensor.matmul(ps1[:], cosv[:, t:t + 1], kt[:, 512:1024],
                                     start=(t == 0), stop=(t == T - 1))

            # drain the two PSUM halves on different engines (parallel tail)
            ctx_sb = pool.tile([1, H], f32, name="ctx_sb")
            nc.scalar.copy(ctx_sb[:, 0:512], ps0[:])
            nc.vector.tensor_copy(ctx_sb[:, 512:1024], ps1[:])
            nc.sync.dma_start(ctx_d[:], ctx_sb[:])

    nc.compile()
    return nc


def _get_nc():
    if "nc" not in _NC_CACHE:
        _NC_CACHE["nc"] = _build_nc()
    return _NC_CACHE["nc"]


def prepare_in_maps(query: np.ndarray, keys: np.ndarray) -> list[dict]:
    import ml_dtypes

    bf16 = ml_dtypes.bfloat16
    query = np.asarray(query, dtype=np.float32)
    keys = np.asarray(keys, dtype=np.float32)
    assert query.shape == (1, H) and keys.shape == (S_FULL, H)

    q = query.reshape(H).astype(np.float64)
    qn = (q / np.linalg.norm(q)).astype(bf16)
    qb = np.ascontiguousarray(np.broadcast_to(qn[None, :], (P, H)))

    keys_bf = keys.astype(bf16)
    shards = keys_bf.reshape(N_CORES, S, H)
    return [{"keys": np.ascontiguousarray(shards[i]), "qb": qb}
            for i in range(N_CORES)]


def combine_results(results: list[dict]) -> np.ndarray:
    partials = np.stack([results[i]["ctx"][0] for i in range(N_CORES)])
    out = partials.astype(np.float64).sum(axis=0).astype(np.float32)
    return out[None, :]


def kernel(query: np.ndarray, keys: np.ndarray) -> np.ndarray:
    from concourse.bass_utils import run_bass_kernel_spmd

    in_maps = prepare_in_maps(query, keys)
    nc = _get_nc()
    res = run_bass_kernel_spmd(nc, in_maps, list(range(N_CORES)))
    return combine_results(res.results)
